# revision 31
# baseline (speedup 1.0000x reference)
"""Trainium2 Bass kernel for nn_BinLoss (MAS binarization loss).

Algorithm
---------
reference = -sum(log(attn) * hard_alignment) / sum(hard_alignment)

Key identity: the masked log-sum over the backtracked MAS path equals the
forward DP value log_p[out_len-1, in_len-1] (Viterbi property), and
sum(hard) == sum(out_lens).  So no backtracking is needed on device.

Device DP (per core, 4 batch elements, data parallel over 8 cores)
------------------------------------------------------------------
DP over rows t:  lp[t, j] = la[t, j] + max(lp[t-1, j], lp[t-1, j-1])

Columns S=400 split SC=16 per partition over 25 partitions; each batch
element owns a 32-partition quadrant (partitions 25..31 scratch).  Each
partition keeps a K=16-wide left halo of its left neighbour's columns so
the j-1 shift stays in-partition; the halo is refreshed every K steps
with one contiguous stream_shuffle (per-quadrant partition rotate).

K=16 DP steps run as ONE custom DVE instruction ("block op"): 4 groups
of 4 rows.  Within a group, element stream is (w, phase A..D); phase r
computes row r at stages (2r, 2r+1) with lag-1 values relayed through
CURR_ALU_OUT flops.  Every phase writes stage 7's flop, so the state
buffer lpw has an interleaved slot layout per column w:

    slot 4w+0..2 : lp[w-1]   (phases A..C emit the held previous column)
    slot 4w+3    : lp[w]     (phase D emits the fresh row-4 value)

Group g+1 reads in0 = lpw shifted +3 elements (stride 1): at element
(w, A) it sees slot 4w+3 = lp[w], and the stage-0 relay at (w-1, D)
reads slot 4w+2 = lp[w-1].  The loopback through SBUF is safe because
the re-read of a slot trails its write by 4W = 128 cycles, beyond the
write-to-read turnaround; any stale leakage lands in the left halo,
which tolerates corruption by construction (refreshed every K steps,
corruption spreads right at 1 col/step and cannot reach real columns).

Masking is data-driven (host writes into its private shard copy):
  * rows >= out_len          -> attn 1.0  => la 0     (value freezes/creeps)
  * row out_len-1, j!=in-1   -> attn 0.0  => la -inf  (kills all but answer)
  * column pad [400, 512)    -> attn 0.0  => la -inf  (isolates quadrants)
Row 0 masking is the lp init: NEG everywhere except col 0 of each batch.
After the last row every surviving finite cell of a quadrant equals the
answer: free-dim reduce_max + host max over the quadrant extracts it.

The host pre-tiles the input so the la stream is fully contiguous per
instruction (SBUF 16B-cacheline friendly): per 16-row block, element
order is (group, w, phase).  ln(attn) runs on the scalar engine.
"""

import sys

import numpy as np

sys.path.insert(0, "/opt/trn_rl_repo")

B, T, S = 32, 1600, 400
N_CORES = 8
BPC = B // N_CORES  # batch elements per core (4)

SC = 16            # columns per partition
PS = S // SC       # used partitions per batch element (25)
GROUP = 32         # partition quadrant per batch element
K = 16             # halo width == steps per block instruction
W = SC + K         # tile width per partition (32)
GQ = K // 4        # 4-row groups per block instruction
BW = 4 * W         # la elements per group / lpw live slots
LBW = K * W        # la elements per block (4*W*GQ)
NSLOT = 4 * W + 3  # lpw slots incl. 3-element right pad
NB_MAX = (T + 15) // 16      # 100 blocks max
TP = 16 * NB_MAX             # padded host rows (1600)
FLATP = W * TP               # per-partition flat length

R_BLOCKS = 8       # steady-state blocks per DMA/ln chunk (128 rows)
NEG = -1.0e30

_prog_cache = {}


# --------------------------------------------------------------------------
# custom DVE op: one 16-step DP block (4 groups x 4 rows)
# --------------------------------------------------------------------------
def _build_block_uops():
    """Per element (w, phase): phase r in {A..D} computes row r+1 at stages
    (2r, 2r+1); lag-1 (j-1) values relay via CURR_ALU_OUT (same stage,
    previous element).  All phases write stage 7's flop: A..C emit the held
    lp[w-1], D computes and emits lp[w] -- producing the interleaved slot
    layout the next group's in0 (+3 shift) consumes."""
    from concourse.dve_uop import (
        DISABLE,
        ENABLE,
        AluInp,
        AluOp,
        InpSel,
        OutPath,
        OutSel,
        Trigger,
        UopConfig,
        UopDpConfig,
    )

    PREV = AluInp.PREV_ALU_OUT
    CURR = AluInp.CURR_ALU_OUT
    L0 = AluInp.PREV_DELAY_0
    L1 = AluInp.PREV_DELAY_1

    def dp_default():
        return [UopDpConfig() for _ in range(8)]

    seed = UopConfig()
    seed.enable_input(InpSel.SRC_0, 1)
    seed.enable_input(InpSel.SRC_1, 2)
    seed.enable_input(InpSel.MAX_NEG, 3)
    seed.trigger = (Trigger.COUNT, Trigger.NONE, Trigger.NONE)
    seed.repeat_count = 1
    seed.next_uop = (1, 0, 0)
    seed.require_inp0 = DISABLE
    seed.require_inp1 = DISABLE
    seed.datapath_config = dp_default()
    b0 = seed.datapath_config[0]
    # latch MAX_NEG into the stage-0 swap flop (left-boundary lag seed)
    b0.enable_alu(AluOp.BYPASS, AluInp.PREV_DELAY_2, AluInp.PREV_DELAY_2)
    b0.swap_enable = ENABLE
    for k in range(1, 8):
        seed.datapath_config[k].pass_through_alu()

    def phase_uop(phase):
        u = UopConfig()
        u.enable_input(InpSel.SRC_0, 1)
        u.enable_input(InpSel.SRC_1, 2)
        u.trigger = (Trigger.SRC_TENSOR_DONE, Trigger.COUNT, Trigger.NONE)
        u.repeat_count = 1
        u.next_uop = (0, 1 + ((phase + 1) % 4), 0)
        u.require_inp0 = ENABLE
        u.require_inp1 = ENABLE
        d = u.datapath_config = dp_default()
        A, B_, C, D = (phase == 0), (phase == 1), (phase == 2), (phase == 3)
        # stage 0: A reads the swap flop (lag lp[w-1]); B latches it from
        # in0 position 4w+4 = live lp[w] (the junk slot written by (w+1,A)).
        if A:
            d[0].enable_alu(AluOp.MAX, L0, AluInp.CURR_SWAP_OUT)
        else:
            d[0].enable_alu(AluOp.BYPASS, L0, L0)
            if B_:
                d[0].swap_enable = ENABLE
        d[0].pass_through_delay(1)
        if A:
            d[1].enable_alu(AluOp.ADD, PREV, L1)
        else:
            d[1].enable_alu(AluOp.BYPASS, CURR, CURR)
        d[1].pass_through_delay(1)
        if A:
            d[2].enable_alu(AluOp.BYPASS, CURR, CURR)
        elif B_:
            d[2].enable_alu(AluOp.MAX, PREV, CURR)
        else:
            d[2].enable_alu(AluOp.BYPASS, PREV, PREV)
        d[2].pass_through_delay(1)
        if B_:
            d[3].enable_alu(AluOp.ADD, PREV, L1)
        else:
            d[3].enable_alu(AluOp.BYPASS, CURR, CURR)
        d[3].pass_through_delay(1)
        if C:
            d[4].enable_alu(AluOp.MAX, PREV, CURR)
        elif D:
            d[4].enable_alu(AluOp.BYPASS, PREV, PREV)
        else:
            d[4].enable_alu(AluOp.BYPASS, CURR, CURR)
        d[4].pass_through_delay(1)
        if C:
            d[5].enable_alu(AluOp.ADD, PREV, L1)
        else:
            d[5].enable_alu(AluOp.BYPASS, CURR, CURR)
        d[5].pass_through_delay(1)
        if D:
            d[6].enable_alu(AluOp.MAX, PREV, CURR)
        elif A:
            d[6].enable_alu(AluOp.BYPASS, PREV, PREV)
        else:
            d[6].enable_alu(AluOp.BYPASS, CURR, CURR)
        d[6].pass_through_delay(1)
        if D:
            d[7].enable_alu(AluOp.ADD, PREV, L1)
        else:
            d[7].enable_alu(AluOp.BYPASS, CURR, CURR)
        u.enable_output(OutSel.ALU_OUT, OutPath.WR0_LO)
        return u

    return [seed] + [phase_uop(p) for p in range(4)]


class _CustomOp:
    subdim = False

    def __init__(self, name, build):
        from concourse.dve_spec import Spec, Src0, Src1

        self.name = name
        self._build = build
        self.spec = Spec(body=Src0 + Src1, reference=None)
        self._cache = {}

    def compile(self, ver):
        from concourse.dve_uop import DveOpSpec

        if ver not in self._cache:
            from concourse.dve_ops import get_dve_sub_opcode

            self._cache[ver] = DveOpSpec(
                name=self.name,
                opcode=get_dve_sub_opcode(self.name),
                uops=self._build(),
                rd1_en=True,
            )
        return self._cache[ver]


def _register_op(name, build):
    import concourse.dve_ops as dve_ops

    for o in dve_ops.OPS:
        if o.name == name:
            return o
    op = _CustomOp(name, build)
    dve_ops.OPS.append(op)
    dve_ops._SUB_OPCODE_FOR_NAME[name] = (
        max(dve_ops._SUB_OPCODE_FOR_NAME.values()) + 1
    )
    assert dve_ops._SUB_OPCODE_FOR_NAME[name] < 0x20
    return op


def _get_block_op():
    return _register_op("MAS_BLOCK_ANT", _build_block_uops)


# --------------------------------------------------------------------------
# program
# --------------------------------------------------------------------------
def _chunk_plan(nb):
    """Chunk sizes in 16-row blocks; fine-grained ramp so the DMA->Ln->DP
    pipeline primes without stalling the vector engine."""
    plan = []
    b0 = 0
    for n in (1, 1, 1, 2, 2, 3, 4, 4, 6, 6):
        if b0 >= nb:
            return plan
        n = min(n, nb - b0)
        plan.append((b0, n))
        b0 += n
    while b0 < nb:
        n = min(R_BLOCKS, nb - b0)
        plan.append((b0, n))
        b0 += n
    return plan


def _build_program(nb):
    import concourse.bacc as bacc
    import concourse.bass as bass
    import concourse.mybir as mybir
    from concourse.tile import TileContext

    opb = _get_block_op()
    f32 = mybir.dt.float32
    bf16 = mybir.dt.bfloat16
    nc = bacc.Bacc("TRN2", target_bir_lowering=False, debug=False)
    attn_d = nc.dram_tensor("attn", [128 * FLATP], bf16, kind="ExternalInput")
    out_d = nc.dram_tensor("res", [128, 1], f32, kind="ExternalOutput")

    shuffle_mask = [31] + list(range(31))  # dest p <- src p-1 within quadrant
    max_chunk = max(n for _, n in _chunk_plan(nb))

    with TileContext(nc) as tc:
        with (
            tc.tile_pool(name="la", bufs=3) as lap,
            tc.tile_pool(name="state", bufs=1) as sp,
        ):
            lpw = sp.tile([128, NSLOT], f32, tag="lpw")
            res = sp.tile([128, 1], f32, tag="res")

            # in0: [[0, GQ], [1, BW]] at +3 elements -- re-walk the live
            # slots once per group, shifted so position 4w reads slot 4w+3.
            in0_ap = lpw[:, 3 : 3 + BW].unsqueeze(1).broadcast_to([128, GQ, BW])
            out_ap = lpw[:, 0:BW].unsqueeze(1).broadcast_to([128, GQ, BW])

            for ci, (b0, nblk) in enumerate(_chunk_plan(nb)):
                ab = lap.tile([128, max_chunk * LBW], bf16, tag="ab")
                la = lap.tile([128, max_chunk * LBW], f32, tag="la")
                nc.sync.dma_start(
                    out=ab[:, 0 : nblk * LBW],
                    in_=bass.AP(
                        attn_d, b0 * LBW, [[FLATP, 128], [1, nblk * LBW]]
                    ),
                )
                nc.scalar.activation(
                    la[:, 0 : nblk * LBW],
                    ab[:, 0 : nblk * LBW],
                    mybir.ActivationFunctionType.Ln,
                )
                if ci == 0:
                    # init: virtual row -1 has lp = 0 at col 0 (position
                    # w=K), -inf elsewhere; block 0's first step then
                    # computes row 0 (host masks row-0 cols >= 1 to 0.0).
                    # The pre-block-0 shuffle below propagates the seed
                    # into partition p+1's halo pair (3, 4).
                    nc.vector.memset(lpw[:, :], NEG)
                    for b in range(BPC):
                        p = GROUP * b
                        nc.vector.memset(
                            lpw[p : p + 1, 4 * K + 3 : 4 * K + 5], 0.0
                        )
                for j in range(nblk):
                    # halo refresh rotate: contiguous dest [3, 4K+3) <-
                    # src live slot 4(W-K+m)+3 duplicated 4x (covers the
                    # live slot 4m+3, lag-source 4m+4, and two junk
                    # slots).  Before block 0 this doubles as the halo
                    # init (it spreads the col-0 seed into partition p+1).
                    pdim = [NSLOT, 128]
                    nc.vector.stream_shuffle(
                        bass.AP(
                            lpw[:, 0:1].tensor, 3, [pdim, [4, K], [1, 4]]
                        ),
                        bass.AP(
                            lpw[:, 0:1].tensor,
                            4 * (W - K) + 3,
                            [pdim, [4, K], [0, 4]],
                        ),
                        mask=shuffle_mask,
                    )
                    base = j * LBW
                    nc.vector._custom_dve(
                        opb,
                        out=out_ap,
                        in0=in0_ap,
                        in1=la[:, base : base + LBW],
                    )

            # real-column slots only: halo slots may hold inflated garbage
            # (absorbed corruption) by design.
            nc.vector.reduce_max(
                res[:, 0:1], lpw[:, 4 * K + 3 : 4 * W], axis=mybir.AxisListType.X
            )
            nc.sync.dma_start(out=out_d.ap(), in_=res[:, 0:1])

    nc.compile()
    return nc


def _prep_shards(attn, in_lens, out_lens, nb):
    """Per-core masked + pre-tiled flat input buffers.

    Device layout per partition: [row0 (W plain)] + nb blocks of
    (group, w, phase) interleaved rows.  Partition 32b+p covers columns
    p*SC - K + w (0.0 outside [0, 400) -> ln = -inf).  Partitions 25..31
    of each quadrant stay 0.0, keeping quadrants isolated through the
    halo-rotate refresh."""
    tp = 16 * nb
    in_maps = []
    pad = K + S + W  # padded column axis: [-K, S + W)
    for core in range(N_CORES):
        sh = np.zeros((BPC, tp, pad), np.float32)
        sh[:, : min(tp, T), K : K + S] = attn[
            core * BPC : (core + 1) * BPC, 0, : min(tp, T)
        ]
        if tp > T:
            sh[:, T:, K : K + S] = 1.0
        sh[:, 0, K + 1 : K + S] = 0.0  # row 0: only col 0 active
        for b in range(BPC):
            ob = int(out_lens[core * BPC + b])
            ib = int(in_lens[core * BPC + b])
            keep = sh[b, ob - 1, K + ib - 1]
            sh[b, ob - 1, K : K + S] = 0.0   # la -> -inf
            sh[b, ob - 1, K + ib - 1] = keep
            sh[b, ob:, K : K + S] = 1.0      # la -> 0
        flat = np.zeros((128, FLATP), np.float32)  # cast to bf16 at the end
        for b in range(BPC):
            win = np.lib.stride_tricks.sliding_window_view(sh[b], W, axis=1)
            arr = win[:, ::SC, :][:, :PS].transpose(1, 0, 2)  # [PS, tp, W]
            X = arr.reshape(PS, nb, 4, 4, W)
            X = X.transpose(0, 1, 2, 4, 3).reshape(PS, nb * LBW)
            flat[GROUP * b : GROUP * b + PS, 0 : nb * LBW] = X
        import ml_dtypes

        in_maps.append({"attn": flat.ravel().astype(ml_dtypes.bfloat16)})
    return in_maps


def _run(attn, in_lens, out_lens, trace=False):
    from concourse import bass_utils

    tmax = int(np.max(out_lens))
    nb = (tmax + 15) // 16
    if nb not in _prog_cache:
        _prog_cache[nb] = _build_program(nb)
    nc = _prog_cache[nb]
    in_maps = _prep_shards(attn, in_lens, out_lens, nb)
    return bass_utils.run_bass_kernel_spmd(
        nc, in_maps, core_ids=list(range(N_CORES)), trace=trace
    )


def kernel(soft_attention, in_lens, out_lens, _trace=False):
    attn = np.asarray(soft_attention, dtype=np.float32)
    inl = np.asarray(in_lens)
    outl = np.asarray(out_lens)
    assert attn.shape == (B, 1, T, S), attn.shape

    res = _run(attn, inl, outl, trace=_trace)

    total = 0.0
    for core in range(N_CORES):
        v = res.results[core]["res"][:, 0]
        for b in range(BPC):
            total += float(np.max(v[GROUP * b : GROUP * b + PS]))
    count = float(np.sum(outl))
    out = np.array(-total / count, dtype=np.float32)
    if _trace:
        return out, res
    return out


# revision 37
# speedup vs baseline: 1.0204x; 1.0204x over previous
"""Trainium2 Bass kernel for nn_BinLoss (MAS binarization loss).

Algorithm
---------
reference = -sum(log(attn) * hard_alignment) / sum(hard_alignment)

Key identity: the masked log-sum over the backtracked MAS path equals the
forward DP value log_p[out_len-1, in_len-1] (Viterbi property), and
sum(hard) == sum(out_lens).  So no backtracking is needed on device.

Device DP (per core, 4 batch elements, data parallel over 8 cores)
------------------------------------------------------------------
DP over rows t:  lp[t, j] = la[t, j] + max(lp[t-1, j], lp[t-1, j-1])

Columns S=400 split SC=16 per partition over 25 partitions; each batch
element owns a 32-partition quadrant (partitions 25..31 scratch).  Each
partition keeps a K=16-wide left halo of its left neighbour's columns so
the j-1 shift stays in-partition; the halo is refreshed every K steps
with one contiguous stream_shuffle (per-quadrant partition rotate).

K=16 DP steps run as ONE custom DVE instruction ("block op"): 4 groups
of 4 rows.  Within a group, element stream is (w, phase A..D); phase r
computes row r at stages (2r, 2r+1) with lag-1 values relayed through
CURR_ALU_OUT flops.  Every phase writes stage 7's flop, so the state
buffer lpw has an interleaved slot layout per column w:

    slot 4w+0..2 : lp[w-1]   (phases A..C emit the held previous column)
    slot 4w+3    : lp[w]     (phase D emits the fresh row-4 value)

Group g+1 reads in0 = lpw shifted +3 elements (stride 1): at element
(w, A) it sees slot 4w+3 = lp[w], and the stage-0 relay at (w-1, D)
reads slot 4w+2 = lp[w-1].  The loopback through SBUF is safe because
the re-read of a slot trails its write by 4W = 128 cycles, beyond the
write-to-read turnaround; any stale leakage lands in the left halo,
which tolerates corruption by construction (refreshed every K steps,
corruption spreads right at 1 col/step and cannot reach real columns).

Masking is data-driven (host writes into its private shard copy):
  * rows >= out_len          -> attn 1.0  => la 0     (value freezes/creeps)
  * row out_len-1, j!=in-1   -> attn 0.0  => la -inf  (kills all but answer)
  * column pad [400, 512)    -> attn 0.0  => la -inf  (isolates quadrants)
Row 0 masking is the lp init: NEG everywhere except col 0 of each batch.
After the last row every surviving finite cell of a quadrant equals the
answer: free-dim reduce_max + host max over the quadrant extracts it.

The host pre-tiles the input so the la stream is fully contiguous per
instruction (SBUF 16B-cacheline friendly): per 16-row block, element
order is (group, w, phase).  ln(attn) runs on the scalar engine.
"""

import sys

import numpy as np

sys.path.insert(0, "/opt/trn_rl_repo")

B, T, S = 32, 1600, 400
N_CORES = 8
BPC = B // N_CORES  # batch elements per core (4)

SC = 16            # columns per partition
PS = S // SC       # used partitions per batch element (25)
GROUP = 32         # partition quadrant per batch element
K = 16             # halo width == steps per block instruction
W = SC + K         # tile width per partition (32)
GQ = K // 4        # 4-row groups per block instruction
BW = 4 * W         # la elements per group / lpw live slots
LBW = K * W        # la elements per block (4*W*GQ)
NSLOT = 4 * W + 3  # lpw slots incl. 3-element right pad
NB_MAX = (T - 1 + 15) // 16  # 100 blocks max
TP = 1 + 16 * NB_MAX         # padded host rows (1601)
FLATP = W * TP               # per-partition flat length

R_BLOCKS = 8       # steady-state blocks per DMA/ln chunk (128 rows)
NEG = -1.0e30

_prog_cache = {}


# --------------------------------------------------------------------------
# custom DVE op: one 16-step DP block (4 groups x 4 rows)
# --------------------------------------------------------------------------
def _build_block_uops():
    """Per element (w, phase): phase r in {A..D} computes row r+1 at stages
    (2r, 2r+1); lag-1 (j-1) values relay via CURR_ALU_OUT (same stage,
    previous element).  All phases write stage 7's flop: A..C emit the held
    lp[w-1], D computes and emits lp[w] -- producing the interleaved slot
    layout the next group's in0 (+3 shift) consumes."""
    from concourse.dve_uop import (
        DISABLE,
        ENABLE,
        AluInp,
        AluOp,
        InpSel,
        OutPath,
        OutSel,
        Trigger,
        UopConfig,
        UopDpConfig,
    )

    PREV = AluInp.PREV_ALU_OUT
    CURR = AluInp.CURR_ALU_OUT
    L0 = AluInp.PREV_DELAY_0
    L1 = AluInp.PREV_DELAY_1

    def dp_default():
        return [UopDpConfig() for _ in range(8)]

    seed = UopConfig()
    seed.enable_input(InpSel.SRC_0, 1)
    seed.enable_input(InpSel.SRC_1, 2)
    seed.enable_input(InpSel.MAX_NEG, 3)
    seed.trigger = (Trigger.COUNT, Trigger.NONE, Trigger.NONE)
    seed.repeat_count = 1
    seed.next_uop = (1, 0, 0)
    seed.require_inp0 = DISABLE
    seed.require_inp1 = DISABLE
    seed.datapath_config = dp_default()
    b0 = seed.datapath_config[0]
    # latch MAX_NEG into the stage-0 swap flop (left-boundary lag seed)
    b0.enable_alu(AluOp.BYPASS, AluInp.PREV_DELAY_2, AluInp.PREV_DELAY_2)
    b0.swap_enable = ENABLE
    for k in range(1, 8):
        seed.datapath_config[k].pass_through_alu()

    def phase_uop(phase):
        u = UopConfig()
        u.enable_input(InpSel.SRC_0, 1)
        u.enable_input(InpSel.SRC_1, 2)
        u.trigger = (Trigger.SRC_TENSOR_DONE, Trigger.COUNT, Trigger.NONE)
        u.repeat_count = 1
        u.next_uop = (0, 1 + ((phase + 1) % 4), 0)
        u.require_inp0 = ENABLE
        u.require_inp1 = ENABLE
        d = u.datapath_config = dp_default()
        A, B_, C, D = (phase == 0), (phase == 1), (phase == 2), (phase == 3)
        # stage 0: A reads the swap flop (lag lp[w-1]); B latches it from
        # in0 position 4w+4 = live lp[w] (the junk slot written by (w+1,A)).
        if A:
            d[0].enable_alu(AluOp.MAX, L0, AluInp.CURR_SWAP_OUT)
        else:
            d[0].enable_alu(AluOp.BYPASS, L0, L0)
            if B_:
                d[0].swap_enable = ENABLE
        d[0].pass_through_delay(1)
        if A:
            d[1].enable_alu(AluOp.ADD, PREV, L1)
        else:
            d[1].enable_alu(AluOp.BYPASS, CURR, CURR)
        d[1].pass_through_delay(1)
        if A:
            d[2].enable_alu(AluOp.BYPASS, CURR, CURR)
        elif B_:
            d[2].enable_alu(AluOp.MAX, PREV, CURR)
        else:
            d[2].enable_alu(AluOp.BYPASS, PREV, PREV)
        d[2].pass_through_delay(1)
        if B_:
            d[3].enable_alu(AluOp.ADD, PREV, L1)
        else:
            d[3].enable_alu(AluOp.BYPASS, CURR, CURR)
        d[3].pass_through_delay(1)
        if C:
            d[4].enable_alu(AluOp.MAX, PREV, CURR)
        elif D:
            d[4].enable_alu(AluOp.BYPASS, PREV, PREV)
        else:
            d[4].enable_alu(AluOp.BYPASS, CURR, CURR)
        d[4].pass_through_delay(1)
        if C:
            d[5].enable_alu(AluOp.ADD, PREV, L1)
        else:
            d[5].enable_alu(AluOp.BYPASS, CURR, CURR)
        d[5].pass_through_delay(1)
        if D:
            d[6].enable_alu(AluOp.MAX, PREV, CURR)
        elif A:
            d[6].enable_alu(AluOp.BYPASS, PREV, PREV)
        else:
            d[6].enable_alu(AluOp.BYPASS, CURR, CURR)
        d[6].pass_through_delay(1)
        if D:
            d[7].enable_alu(AluOp.ADD, PREV, L1)
        else:
            d[7].enable_alu(AluOp.BYPASS, CURR, CURR)
        u.enable_output(OutSel.ALU_OUT, OutPath.WR0_LO)
        return u

    return [seed] + [phase_uop(p) for p in range(4)]


class _CustomOp:
    subdim = False

    def __init__(self, name, build):
        from concourse.dve_spec import Spec, Src0, Src1

        self.name = name
        self._build = build
        self.spec = Spec(body=Src0 + Src1, reference=None)
        self._cache = {}

    def compile(self, ver):
        from concourse.dve_uop import DveOpSpec

        if ver not in self._cache:
            from concourse.dve_ops import get_dve_sub_opcode

            self._cache[ver] = DveOpSpec(
                name=self.name,
                opcode=get_dve_sub_opcode(self.name),
                uops=self._build(),
                rd1_en=True,
            )
        return self._cache[ver]


def _register_op(name, build):
    import concourse.dve_ops as dve_ops

    for o in dve_ops.OPS:
        if o.name == name:
            return o
    op = _CustomOp(name, build)
    dve_ops.OPS.append(op)
    dve_ops._SUB_OPCODE_FOR_NAME[name] = (
        max(dve_ops._SUB_OPCODE_FOR_NAME.values()) + 1
    )
    assert dve_ops._SUB_OPCODE_FOR_NAME[name] < 0x20
    return op


def _get_block_op():
    return _register_op("MAS_BLOCK_ANT", _build_block_uops)


# --------------------------------------------------------------------------
# program
# --------------------------------------------------------------------------
def _chunk_plan(nb):
    """Chunk sizes in 16-row blocks; fine-grained ramp so the DMA->Ln->DP
    pipeline primes without stalling the vector engine."""
    plan = []
    b0 = 0
    for n in (1, 1, 1, 2, 2, 3, 4, 4, 6, 6):
        if b0 >= nb:
            return plan
        n = min(n, nb - b0)
        plan.append((b0, n))
        b0 += n
    while b0 < nb:
        n = min(R_BLOCKS, nb - b0)
        plan.append((b0, n))
        b0 += n
    return plan


def _build_program(nb):
    import concourse.bacc as bacc
    import concourse.bass as bass
    import concourse.mybir as mybir
    from concourse.tile import TileContext

    opb = _get_block_op()
    f32 = mybir.dt.float32
    bf16 = mybir.dt.bfloat16
    nc = bacc.Bacc("TRN2", target_bir_lowering=False, debug=False)
    attn_d = nc.dram_tensor("attn", [128 * FLATP], bf16, kind="ExternalInput")
    out_d = nc.dram_tensor("res", [128, 1], f32, kind="ExternalOutput")

    shuffle_mask = [31] + list(range(31))  # dest p <- src p-1 within quadrant
    max_chunk = max(n for _, n in _chunk_plan(nb))

    with TileContext(nc) as tc:
        with (
            tc.tile_pool(name="la", bufs=3) as lap,
            tc.tile_pool(name="state", bufs=1) as sp,
        ):
            lpw = sp.tile([128, NSLOT], f32, tag="lpw")
            res = sp.tile([128, 1], f32, tag="res")

            # in0: [[0, GQ], [1, BW]] at +3 elements -- re-walk the live
            # slots once per group, shifted so position 4w reads slot 4w+3.
            in0_ap = lpw[:, 3 : 3 + BW].unsqueeze(1).broadcast_to([128, GQ, BW])
            out_ap = lpw[:, 0:BW].unsqueeze(1).broadcast_to([128, GQ, BW])

            for ci, (b0, nblk) in enumerate(_chunk_plan(nb)):
                extra = W if ci == 0 else 0  # chunk 0 carries row 0
                ab = lap.tile([128, W + max_chunk * LBW], bf16, tag="ab")
                la = lap.tile([128, W + max_chunk * LBW], f32, tag="la")
                off_d = (W + b0 * LBW) - extra
                nc.sync.dma_start(
                    out=ab[:, 0 : extra + nblk * LBW],
                    in_=bass.AP(
                        attn_d, off_d, [[FLATP, 128], [1, extra + nblk * LBW]]
                    ),
                )
                if ci == 0:
                    # row 0 first: the init copies only need these W
                    # elements, so they start before the block-0 Ln.
                    nc.scalar.activation(
                        la[:, 0:W], ab[:, 0:W],
                        mybir.ActivationFunctionType.Ln,
                    )
                    nc.scalar.activation(
                        la[:, W : extra + nblk * LBW],
                        ab[:, W : extra + nblk * LBW],
                        mybir.ActivationFunctionType.Ln,
                    )
                else:
                    nc.scalar.activation(
                        la[:, 0 : nblk * LBW],
                        ab[:, 0 : nblk * LBW],
                        mybir.ActivationFunctionType.Ln,
                    )
                if ci == 0:
                    nc.vector.memset(lpw[:, :], NEG)
                    for b in range(BPC):
                        p = GROUP * b
                        # lp[col 0] = la_row0[col 0]; col 0 sits at w=K.
                        # Live slot 4w+3 and lag-source slot 4w+4.  The
                        # pre-block-0 shuffle below propagates the live
                        # slot into partition p+1's halo pair (3, 4).
                        nc.vector.tensor_copy(
                            lpw[p : p + 1, 4 * K + 3 : 4 * K + 5],
                            la[p : p + 1, K : K + 1].broadcast_to([1, 2]),
                        )
                for j in range(nblk):
                    # halo refresh rotate: contiguous dest [3, 4K+3) <-
                    # src live slot 4(W-K+m)+3 duplicated 4x (covers the
                    # live slot 4m+3, lag-source 4m+4, and two junk
                    # slots).  Before block 0 this doubles as the halo
                    # init (it spreads the col-0 seed into partition p+1).
                    pdim = [NSLOT, 128]
                    nc.vector.stream_shuffle(
                        bass.AP(
                            lpw[:, 0:1].tensor, 3, [pdim, [4, K], [1, 4]]
                        ),
                        bass.AP(
                            lpw[:, 0:1].tensor,
                            4 * (W - K) + 3,
                            [pdim, [4, K], [0, 4]],
                        ),
                        mask=shuffle_mask,
                    )
                    base = extra + j * LBW
                    nc.vector._custom_dve(
                        opb,
                        out=out_ap,
                        in0=in0_ap,
                        in1=la[:, base : base + LBW],
                    )

            # real-column slots only: halo slots may hold inflated garbage
            # (absorbed corruption) by design.
            nc.vector.reduce_max(
                res[:, 0:1], lpw[:, 4 * K + 3 : 4 * W], axis=mybir.AxisListType.X
            )
            nc.sync.dma_start(out=out_d.ap(), in_=res[:, 0:1])

    nc.compile()
    return nc


def _prep_shards(attn, in_lens, out_lens, nb):
    """Per-core masked + pre-tiled flat input buffers.

    Device layout per partition: [row0 (W plain)] + nb blocks of
    (group, w, phase) interleaved rows.  Partition 32b+p covers columns
    p*SC - K + w (0.0 outside [0, 400) -> ln = -inf).  Partitions 25..31
    of each quadrant stay 0.0, keeping quadrants isolated through the
    halo-rotate refresh."""
    tp = 1 + 16 * nb
    in_maps = []
    pad = K + S + W  # padded column axis: [-K, S + W)
    for core in range(N_CORES):
        sh = np.zeros((BPC, tp, pad), np.float32)
        sh[:, : min(tp, T), K : K + S] = attn[
            core * BPC : (core + 1) * BPC, 0, : min(tp, T)
        ]
        if tp > T:
            sh[:, T:, K : K + S] = 1.0
        for b in range(BPC):
            ob = int(out_lens[core * BPC + b])
            ib = int(in_lens[core * BPC + b])
            keep = sh[b, ob - 1, K + ib - 1]
            sh[b, ob - 1, K : K + S] = 0.0   # la -> -inf
            sh[b, ob - 1, K + ib - 1] = keep
            sh[b, ob:, K : K + S] = 1.0      # la -> 0
        flat = np.zeros((128, FLATP), np.float32)  # cast to bf16 at the end
        for b in range(BPC):
            win = np.lib.stride_tricks.sliding_window_view(sh[b], W, axis=1)
            arr = win[:, ::SC, :][:, :PS].transpose(1, 0, 2)  # [PS, tp, W]
            flat[GROUP * b : GROUP * b + PS, 0:W] = arr[:, 0, :]
            X = arr[:, 1 : 1 + 16 * nb, :].reshape(PS, nb, 4, 4, W)
            X = X.transpose(0, 1, 2, 4, 3).reshape(PS, nb * LBW)
            flat[GROUP * b : GROUP * b + PS, W : W + nb * LBW] = X
        import ml_dtypes

        in_maps.append({"attn": flat.ravel().astype(ml_dtypes.bfloat16)})
    return in_maps


def _run(attn, in_lens, out_lens, trace=False):
    from concourse import bass_utils

    tmax = int(np.max(out_lens))
    nb = (tmax - 1 + 15) // 16
    if nb not in _prog_cache:
        _prog_cache[nb] = _build_program(nb)
    nc = _prog_cache[nb]
    in_maps = _prep_shards(attn, in_lens, out_lens, nb)
    return bass_utils.run_bass_kernel_spmd(
        nc, in_maps, core_ids=list(range(N_CORES)), trace=trace
    )


def kernel(soft_attention, in_lens, out_lens, _trace=False):
    attn = np.asarray(soft_attention, dtype=np.float32)
    inl = np.asarray(in_lens)
    outl = np.asarray(out_lens)
    assert attn.shape == (B, 1, T, S), attn.shape

    res = _run(attn, inl, outl, trace=_trace)

    total = 0.0
    for core in range(N_CORES):
        v = res.results[core]["res"][:, 0]
        for b in range(BPC):
            total += float(np.max(v[GROUP * b : GROUP * b + PS]))
    count = float(np.sum(outl))
    out = np.array(-total / count, dtype=np.float32)
    if _trace:
        return out, res
    return out


# revision 40
# speedup vs baseline: 1.0772x; 1.0558x over previous
"""Trainium2 Bass kernel for nn_BinLoss (MAS binarization loss).

Algorithm
---------
reference = -sum(log(attn) * hard_alignment) / sum(hard_alignment)

Key identity: the masked log-sum over the backtracked MAS path equals the
forward DP value log_p[out_len-1, in_len-1] (Viterbi property), and
sum(hard) == sum(out_lens).  So no backtracking is needed on device.

Device DP (per core, 4 batch elements, data parallel over 8 cores)
------------------------------------------------------------------
DP over rows t:  lp[t, j] = la[t, j] + max(lp[t-1, j], lp[t-1, j-1])

Columns S=400 split SC=16 per partition over 25 partitions; each batch
element owns a 32-partition quadrant (partitions 25..31 scratch).  Each
partition keeps a K=16-wide left halo of its left neighbour's columns so
the j-1 shift stays in-partition; the halo is refreshed every K steps
with one contiguous stream_shuffle (per-quadrant partition rotate).

K=16 DP steps run as ONE custom DVE instruction ("block op"): 4 groups
of 4 rows.  Within a group, element stream is (w, phase A..D); phase r
computes row r at stages (2r, 2r+1) with lag-1 values relayed through
CURR_ALU_OUT flops.  Every phase writes stage 7's flop, so the state
buffer lpw has an interleaved slot layout per column w:

    slot 4w+0..2 : lp[w-1]   (phases A..C emit the held previous column)
    slot 4w+3    : lp[w]     (phase D emits the fresh row-4 value)

Group g+1 reads in0 = lpw shifted +3 elements (stride 1): at element
(w, A) it sees slot 4w+3 = lp[w], and the stage-0 relay at (w-1, D)
reads slot 4w+2 = lp[w-1].  The loopback through SBUF is safe because
the re-read of a slot trails its write by 4W = 128 cycles, beyond the
write-to-read turnaround; any stale leakage lands in the left halo,
which tolerates corruption by construction (refreshed every K steps,
corruption spreads right at 1 col/step and cannot reach real columns).

Masking is data-driven (host writes into its private shard copy):
  * rows >= out_len          -> attn 1.0  => la 0     (value freezes/creeps)
  * row out_len-1, j!=in-1   -> attn 0.0  => la -inf  (kills all but answer)
  * column pad [400, 512)    -> attn 0.0  => la -inf  (isolates quadrants)
Row 0 masking is the lp init: NEG everywhere except col 0 of each batch.
After the last row every surviving finite cell of a quadrant equals the
answer: free-dim reduce_max + host max over the quadrant extracts it.

The host pre-tiles the input so the la stream is fully contiguous per
instruction (SBUF 16B-cacheline friendly): per 16-row block, element
order is (group, w, phase).  ln(attn) runs on the scalar engine.
"""

import sys

import numpy as np

sys.path.insert(0, "/opt/trn_rl_repo")

B, T, S = 32, 1600, 400
N_CORES = 8
BPC = B // N_CORES  # batch elements per core (4)

SC = 16            # columns per partition
PS = S // SC       # used partitions per batch element (25)
GROUP = 32         # partition quadrant per batch element
K = 16             # halo width == steps per block instruction
W = SC + K         # tile width per partition (32)
GQ = K // 4        # 4-row groups per block instruction
BW = 4 * W         # la elements per group / lpw live slots
LBW = K * W        # la elements per block (4*W*GQ)
NSLOT = 4 * W + 3  # lpw slots incl. 3-element right pad
NB_MAX = (T - 1 + 15) // 16  # 100 blocks max
TP = 1 + 16 * NB_MAX         # padded host rows (1601)
FLATP = W * TP               # per-partition flat length

R_BLOCKS = 8       # steady-state blocks per DMA/ln chunk (128 rows)
NEG = -1.0e30

_prog_cache = {}


# --------------------------------------------------------------------------
# custom DVE op: one 16-step DP block (4 groups x 4 rows)
# --------------------------------------------------------------------------
def _build_block_uops():
    """Per element (w, phase): phase r in {A..D} computes row r+1 at stages
    (2r, 2r+1); lag-1 (j-1) values relay via CURR_ALU_OUT (same stage,
    previous element).  All phases write stage 7's flop: A..C emit the held
    lp[w-1], D computes and emits lp[w] -- producing the interleaved slot
    layout the next group's in0 (+3 shift) consumes."""
    from concourse.dve_uop import (
        DISABLE,
        ENABLE,
        AluInp,
        AluOp,
        InpSel,
        OutPath,
        OutSel,
        Trigger,
        UopConfig,
        UopDpConfig,
    )

    PREV = AluInp.PREV_ALU_OUT
    CURR = AluInp.CURR_ALU_OUT
    L0 = AluInp.PREV_DELAY_0
    L1 = AluInp.PREV_DELAY_1

    def dp_default():
        return [UopDpConfig() for _ in range(8)]

    seed = UopConfig()
    seed.enable_input(InpSel.SRC_0, 1)
    seed.enable_input(InpSel.SRC_1, 2)
    seed.enable_input(InpSel.MAX_NEG, 3)
    seed.trigger = (Trigger.COUNT, Trigger.NONE, Trigger.NONE)
    seed.repeat_count = 1
    seed.next_uop = (1, 0, 0)
    seed.require_inp0 = DISABLE
    seed.require_inp1 = DISABLE
    seed.datapath_config = dp_default()
    b0 = seed.datapath_config[0]
    # latch MAX_NEG into the stage-0 swap flop (left-boundary lag seed)
    b0.enable_alu(AluOp.BYPASS, AluInp.PREV_DELAY_2, AluInp.PREV_DELAY_2)
    b0.swap_enable = ENABLE
    for k in range(1, 8):
        seed.datapath_config[k].pass_through_alu()

    def phase_uop(phase):
        u = UopConfig()
        u.enable_input(InpSel.SRC_0, 1)
        u.enable_input(InpSel.SRC_1, 2)
        u.trigger = (Trigger.SRC_TENSOR_DONE, Trigger.COUNT, Trigger.NONE)
        u.repeat_count = 1
        u.next_uop = (0, 1 + ((phase + 1) % 4), 0)
        u.require_inp0 = ENABLE
        u.require_inp1 = ENABLE
        d = u.datapath_config = dp_default()
        A, B_, C, D = (phase == 0), (phase == 1), (phase == 2), (phase == 3)
        # stage 0: A reads the swap flop (lag lp[w-1]); B latches it from
        # in0 position 4w+4 = live lp[w] (the junk slot written by (w+1,A)).
        if A:
            d[0].enable_alu(AluOp.MAX, L0, AluInp.CURR_SWAP_OUT)
        else:
            d[0].enable_alu(AluOp.BYPASS, L0, L0)
            if B_:
                d[0].swap_enable = ENABLE
        d[0].pass_through_delay(1)
        if A:
            d[1].enable_alu(AluOp.ADD, PREV, L1)
        else:
            d[1].enable_alu(AluOp.BYPASS, CURR, CURR)
        d[1].pass_through_delay(1)
        if A:
            d[2].enable_alu(AluOp.BYPASS, CURR, CURR)
        elif B_:
            d[2].enable_alu(AluOp.MAX, PREV, CURR)
        else:
            d[2].enable_alu(AluOp.BYPASS, PREV, PREV)
        d[2].pass_through_delay(1)
        if B_:
            d[3].enable_alu(AluOp.ADD, PREV, L1)
        else:
            d[3].enable_alu(AluOp.BYPASS, CURR, CURR)
        d[3].pass_through_delay(1)
        if C:
            d[4].enable_alu(AluOp.MAX, PREV, CURR)
        elif D:
            d[4].enable_alu(AluOp.BYPASS, PREV, PREV)
        else:
            d[4].enable_alu(AluOp.BYPASS, CURR, CURR)
        d[4].pass_through_delay(1)
        if C:
            d[5].enable_alu(AluOp.ADD, PREV, L1)
        else:
            d[5].enable_alu(AluOp.BYPASS, CURR, CURR)
        d[5].pass_through_delay(1)
        if D:
            d[6].enable_alu(AluOp.MAX, PREV, CURR)
        elif A:
            d[6].enable_alu(AluOp.BYPASS, PREV, PREV)
        else:
            d[6].enable_alu(AluOp.BYPASS, CURR, CURR)
        d[6].pass_through_delay(1)
        if D:
            d[7].enable_alu(AluOp.ADD, PREV, L1)
        else:
            d[7].enable_alu(AluOp.BYPASS, CURR, CURR)
        u.enable_output(OutSel.ALU_OUT, OutPath.WR0_LO)
        return u

    return [seed] + [phase_uop(p) for p in range(4)]


class _CustomOp:
    subdim = False

    def __init__(self, name, build):
        from concourse.dve_spec import Spec, Src0, Src1

        self.name = name
        self._build = build
        self.spec = Spec(body=Src0 + Src1, reference=None)
        self._cache = {}

    def compile(self, ver):
        from concourse.dve_uop import DveOpSpec

        if ver not in self._cache:
            from concourse.dve_ops import get_dve_sub_opcode

            self._cache[ver] = DveOpSpec(
                name=self.name,
                opcode=get_dve_sub_opcode(self.name),
                uops=self._build(),
                rd1_en=True,
            )
        return self._cache[ver]


def _register_op(name, build):
    import concourse.dve_ops as dve_ops

    for o in dve_ops.OPS:
        if o.name == name:
            return o
    op = _CustomOp(name, build)
    dve_ops.OPS.append(op)
    dve_ops._SUB_OPCODE_FOR_NAME[name] = (
        max(dve_ops._SUB_OPCODE_FOR_NAME.values()) + 1
    )
    assert dve_ops._SUB_OPCODE_FOR_NAME[name] < 0x20
    return op


def _get_block_op():
    return _register_op("MAS_BLOCK_ANT", _build_block_uops)


# --------------------------------------------------------------------------
# program
# --------------------------------------------------------------------------
def _chunk_plan(nb):
    """Chunk sizes in 16-row blocks; fine-grained ramp so the DMA->Ln->DP
    pipeline primes without stalling the vector engine."""
    plan = []
    b0 = 0
    for n in (1, 1, 1, 2, 2, 3, 4, 4, 6, 6):
        if b0 >= nb:
            return plan
        n = min(n, nb - b0)
        plan.append((b0, n))
        b0 += n
    while b0 < nb:
        n = min(R_BLOCKS, nb - b0)
        plan.append((b0, n))
        b0 += n
    return plan


def _build_program(nb):
    import concourse.bacc as bacc
    import concourse.bass as bass
    import concourse.mybir as mybir
    from concourse.tile import TileContext

    opb = _get_block_op()
    f32 = mybir.dt.float32
    bf16 = mybir.dt.bfloat16
    nc = bacc.Bacc("TRN2", target_bir_lowering=False, debug=False)
    attn_d = nc.dram_tensor("attn", [128 * FLATP], bf16, kind="ExternalInput")
    # 64B per partition: sub-burst 4B scattered writes stall the DMA
    # completion path by ~7us at kernel end.
    out_d = nc.dram_tensor("res", [128, 16], f32, kind="ExternalOutput")

    shuffle_mask = [31] + list(range(31))  # dest p <- src p-1 within quadrant
    max_chunk = max(n for _, n in _chunk_plan(nb))

    with TileContext(nc) as tc:
        with (
            tc.tile_pool(name="la", bufs=3) as lap,
            tc.tile_pool(name="state", bufs=1) as sp,
        ):
            lpw = sp.tile([128, NSLOT], f32, tag="lpw")
            res = sp.tile([128, 16], f32, tag="res")

            # in0: [[0, GQ], [1, BW]] at +3 elements -- re-walk the live
            # slots once per group, shifted so position 4w reads slot 4w+3.
            in0_ap = lpw[:, 3 : 3 + BW].unsqueeze(1).broadcast_to([128, GQ, BW])
            out_ap = lpw[:, 0:BW].unsqueeze(1).broadcast_to([128, GQ, BW])

            for ci, (b0, nblk) in enumerate(_chunk_plan(nb)):
                extra = W if ci == 0 else 0  # chunk 0 carries row 0
                ab = lap.tile([128, W + max_chunk * LBW], bf16, tag="ab")
                la = lap.tile([128, W + max_chunk * LBW], f32, tag="la")
                off_d = (W + b0 * LBW) - extra
                nc.sync.dma_start(
                    out=ab[:, 0 : extra + nblk * LBW],
                    in_=bass.AP(
                        attn_d, off_d, [[FLATP, 128], [1, extra + nblk * LBW]]
                    ),
                )
                if ci == 0:
                    # row 0 first: the init copies only need these W
                    # elements, so they start before the block-0 Ln.
                    nc.scalar.activation(
                        la[:, 0:W], ab[:, 0:W],
                        mybir.ActivationFunctionType.Ln,
                    )
                    nc.scalar.activation(
                        la[:, W : extra + nblk * LBW],
                        ab[:, W : extra + nblk * LBW],
                        mybir.ActivationFunctionType.Ln,
                    )
                else:
                    nc.scalar.activation(
                        la[:, 0 : nblk * LBW],
                        ab[:, 0 : nblk * LBW],
                        mybir.ActivationFunctionType.Ln,
                    )
                if ci == 0:
                    nc.vector.memset(lpw[:, :], NEG)
                    for b in range(BPC):
                        p = GROUP * b
                        # lp[col 0] = la_row0[col 0]; col 0 sits at w=K.
                        # Live slot 4w+3 and lag-source slot 4w+4.  The
                        # pre-block-0 shuffle below propagates the live
                        # slot into partition p+1's halo pair (3, 4).
                        nc.vector.tensor_copy(
                            lpw[p : p + 1, 4 * K + 3 : 4 * K + 5],
                            la[p : p + 1, K : K + 1].broadcast_to([1, 2]),
                        )
                for j in range(nblk):
                    # halo refresh rotate: contiguous dest [3, 4K+3) <-
                    # src live slot 4(W-K+m)+3 duplicated 4x (covers the
                    # live slot 4m+3, lag-source 4m+4, and two junk
                    # slots).  Before block 0 this doubles as the halo
                    # init (it spreads the col-0 seed into partition p+1).
                    pdim = [NSLOT, 128]
                    nc.vector.stream_shuffle(
                        bass.AP(
                            lpw[:, 0:1].tensor, 3, [pdim, [4, K], [1, 4]]
                        ),
                        bass.AP(
                            lpw[:, 0:1].tensor,
                            4 * (W - K) + 3,
                            [pdim, [4, K], [0, 4]],
                        ),
                        mask=shuffle_mask,
                    )
                    base = extra + j * LBW
                    nc.vector._custom_dve(
                        opb,
                        out=out_ap,
                        in0=in0_ap,
                        in1=la[:, base : base + LBW],
                    )

            # real-column slots only: halo slots may hold inflated garbage
            # (absorbed corruption) by design.
            nc.vector.reduce_max(
                res[:, 0:1], lpw[:, 4 * K + 3 : 4 * W], axis=mybir.AxisListType.X
            )
            nc.sync.dma_start(out=out_d.ap(), in_=res[:, 0:16])

    nc.compile()
    return nc


def _prep_shards(attn, in_lens, out_lens, nb):
    """Per-core masked + pre-tiled flat input buffers.

    Device layout per partition: [row0 (W plain)] + nb blocks of
    (group, w, phase) interleaved rows.  Partition 32b+p covers columns
    p*SC - K + w (0.0 outside [0, 400) -> ln = -inf).  Partitions 25..31
    of each quadrant stay 0.0, keeping quadrants isolated through the
    halo-rotate refresh."""
    tp = 1 + 16 * nb
    in_maps = []
    pad = K + S + W  # padded column axis: [-K, S + W)
    for core in range(N_CORES):
        sh = np.zeros((BPC, tp, pad), np.float32)
        sh[:, : min(tp, T), K : K + S] = attn[
            core * BPC : (core + 1) * BPC, 0, : min(tp, T)
        ]
        if tp > T:
            sh[:, T:, K : K + S] = 1.0
        for b in range(BPC):
            ob = int(out_lens[core * BPC + b])
            ib = int(in_lens[core * BPC + b])
            keep = sh[b, ob - 1, K + ib - 1]
            sh[b, ob - 1, K : K + S] = 0.0   # la -> -inf
            sh[b, ob - 1, K + ib - 1] = keep
            sh[b, ob:, K : K + S] = 1.0      # la -> 0
        flat = np.zeros((128, FLATP), np.float32)  # cast to bf16 at the end
        for b in range(BPC):
            win = np.lib.stride_tricks.sliding_window_view(sh[b], W, axis=1)
            arr = win[:, ::SC, :][:, :PS].transpose(1, 0, 2)  # [PS, tp, W]
            flat[GROUP * b : GROUP * b + PS, 0:W] = arr[:, 0, :]
            X = arr[:, 1 : 1 + 16 * nb, :].reshape(PS, nb, 4, 4, W)
            X = X.transpose(0, 1, 2, 4, 3).reshape(PS, nb * LBW)
            flat[GROUP * b : GROUP * b + PS, W : W + nb * LBW] = X
        import ml_dtypes

        in_maps.append({"attn": flat.ravel().astype(ml_dtypes.bfloat16)})
    return in_maps


def _run(attn, in_lens, out_lens, trace=False):
    from concourse import bass_utils

    tmax = int(np.max(out_lens))
    nb = (tmax - 1 + 15) // 16
    if nb not in _prog_cache:
        _prog_cache[nb] = _build_program(nb)
    nc = _prog_cache[nb]
    in_maps = _prep_shards(attn, in_lens, out_lens, nb)
    return bass_utils.run_bass_kernel_spmd(
        nc, in_maps, core_ids=list(range(N_CORES)), trace=trace
    )


def kernel(soft_attention, in_lens, out_lens, _trace=False):
    attn = np.asarray(soft_attention, dtype=np.float32)
    inl = np.asarray(in_lens)
    outl = np.asarray(out_lens)
    assert attn.shape == (B, 1, T, S), attn.shape

    res = _run(attn, inl, outl, trace=_trace)

    total = 0.0
    for core in range(N_CORES):
        v = res.results[core]["res"][:, 0]
        for b in range(BPC):
            total += float(np.max(v[GROUP * b : GROUP * b + PS]))
    count = float(np.sum(outl))
    out = np.array(-total / count, dtype=np.float32)
    if _trace:
        return out, res
    return out
